# revision 1
# baseline (speedup 1.0000x reference)
"""DistanceNetwork (retrieval kNN cosine similarity) TRN2 Bass kernel.

reference:
    input_mag = rsqrt(max(sum(input**2), eps))              # global scalar
    support_mag = rsqrt(max(sum(support**2, axis=1), eps))  # [n]
    out[n, b, 0] = dot(support[n], input[b]) * support_mag[n] * input_mag

Shapes (hardcoded): support_set [8192, 1024] f32, input_image [2048, 1024] f32,
out [8192, 2048, 1] f32.

Sharding: support rows split across 8 cores (1024 rows / core); input_image
replicated (each core needs all of it for the global magnitude anyway, and
this halves HBM traffic vs replicating the 32MB support set). No collectives.

Device layout: host pre-transposes both operands so the contraction dim d
lands on SBUF partitions:
    s_t [1024 (d), 1024 (n_shard)]   x_t [1024 (d), 2048 (b)]
Main matmuls run in float32r (full PE rate, ~1.5e-4 scale-relative error).

Engines are strictly in-order, so emission order is chosen to match data
arrival: bt=0 matmuls stream kt-by-kt during the load, per-bt drains are
interleaved with per-bt x^2 squares on ACT, and the global input magnitude
uses GpSimd partition_all_reduce so it never enters the PE stream.
"""

import numpy as np

import concourse.bass as bass
import concourse.bacc as bacc
import concourse.bass_isa as bass_isa
import concourse.tile as tile
import concourse.mybir as mybir
from concourse.bass_utils import run_bass_kernel_spmd

F32 = mybir.dt.float32
F32R = mybir.dt.float32r
AF = mybir.ActivationFunctionType
ALU = mybir.AluOpType

D = 1024          # feature dim (contraction)
NS = 1024         # support rows per core
B = 2048          # query batch (replicated per core)
KT = D // 128     # 8 contraction tiles
NT = NS // 128    # 8 output-partition tiles
BT = B // 512     # 4 moving-dim chunks
EPS = 1e-10
N_CORES = 8


def _newton_rsqrt(nc, pool, a_ap, seed_ap, shape, pfx, iters=2):
    """r ~= rsqrt(a) refined from seed (1/sqrt via LUT) with Newton steps.

    r <- r * (1.5 - 0.5 * a * r * r).  All tiles [P, W] f32.
    """
    r = seed_ap
    for i in range(iters):
        t = pool.tile(shape, F32, tag=f"{pfx}_t{i}", name=f"{pfx}_t{i}")
        nc.vector.tensor_mul(t[:], r, r)
        nc.vector.tensor_mul(t[:], a_ap, t[:])
        nc.vector.tensor_scalar(
            t[:], t[:], -0.5, 1.5, op0=ALU.mult, op1=ALU.add
        )
        r2 = pool.tile(shape, F32, tag=f"{pfx}_r{i}", name=f"{pfx}_r{i}")
        nc.vector.tensor_mul(r2[:], r, t[:])
        r = r2[:]
    return r


def build_nc():
    nc = bacc.Bacc(None, target_bir_lowering=False)
    s_dram = nc.declare_dram_parameter("s_t", [D, NS], F32R, isOutput=False)
    x_dram = nc.declare_dram_parameter("x_t", [D, B], F32R, isOutput=False)
    o_dram = nc.declare_dram_parameter("out", [NS, B], F32, isOutput=True)
    ssq_dram = nc.dram_tensor("ssq_bounce", [NT, 128], F32)

    with tile.TileContext(nc) as tc:
        with (
            tc.tile_pool(name="sp", bufs=KT) as sp,
            tc.tile_pool(name="xp", bufs=KT * BT) as xp,
            tc.tile_pool(name="op", bufs=NT * BT) as op,
            tc.tile_pool(name="s2p", bufs=3) as s2p,
            tc.tile_pool(name="scrp", bufs=3) as scrp,
            tc.tile_pool(name="small", bufs=1) as small,
            tc.tile_pool(name="psum", bufs=8, space="PSUM") as psum,
        ):
            # ---- constants (tiny DMAs issued before the bulk loads) ---------
            ones = small.tile([128, 128], F32)
            nc.vector.memset(ones[:], 1.0)
            # pin ACT's sqrt table set before the Square stream starts, so the
            # mid-kernel Sqrt calls don't force a ~2.7us table reload
            ones_r = small.tile([128, 1], F32R)
            # f32r memset is invalid ISA; byte-copy 1.0f from the f32 ones
            nc.sync.dma_start(out=ones_r[:], in_=ones[:, 0:1].bitcast(F32R))
            sq_dummy = small.tile([1, 1], F32)
            nc.scalar.activation(sq_dummy[:], ones[0:1, 0:1], AF.Sqrt)

            accs = small.tile([128, KT * BT], F32)
            s_sb = [None] * KT
            s2_sb = [None] * KT
            x_sb = [[None] * BT for _ in range(KT)]
            o_sb = [[None] * NT for _ in range(BT)]

            def load_x(kt, bt):
                t = xp.tile([128, 512], F32R, tag="x_sb", name=f"x{kt}_{bt}")
                nc.sync.dma_start(
                    out=t[:],
                    in_=x_dram[kt * 128:(kt + 1) * 128, bt * 512:(bt + 1) * 512],
                )
                x_sb[kt][bt] = t

            def square_x(kt, bt):
                # per-partition sum of x^2 on ACT (TensorTensorReduce faults
                # on HW; ACT Square + free-dim accumulator works)
                scr = scrp.tile([128, 512], F32, tag="scr", name=f"scr{kt}_{bt}")
                nc.scalar.activation(
                    scr[:], x_sb[kt][bt][:].bitcast(F32), AF.Square,
                    accum_out=accs[:, (bt * KT + kt):(bt * KT + kt) + 1],
                )

            # ---- input DMAs: (x bt=0, s) interleaved in 128KB ring slices,
            # then x bt=1..3 ------------------------------------------------
            for kt in range(KT):
                load_x(kt, 0)
                t = sp.tile([128, NS], F32R, tag="s_sb", name=f"s{kt}")
                for q in range(2):
                    nc.sync.dma_start(
                        out=t[:, q * 512:(q + 1) * 512],
                        in_=s_dram[kt * 128:(kt + 1) * 128,
                                   q * 512:(q + 1) * 512],
                    )
                s_sb[kt] = t
                s2 = s2p.tile([128, NS], F32R, tag="s2", name=f"s2_{kt}")
                nc.vector.tensor_mul(s2[:], t[:], t[:])
                s2_sb[kt] = s2
                square_x(kt, 0)
            for bt in range(1, BT):
                for kt in range(KT):
                    load_x(kt, bt)

            def main_mm(ps_ap, kt, nt, bt):
                nc.tensor.matmul(
                    ps_ap,
                    s_sb[kt][:, nt * 128:(nt + 1) * 128],
                    x_sb[kt][bt][:],
                    start=(kt == 0),
                    stop=(kt == KT - 1),
                )

            def drain(bt, nt, ps_ap):
                # plain copy: PSUM frees at PE pace; both magnitude scales are
                # applied in the (already existing) second pass
                o = op.tile([128, 512], F32, tag="o", name=f"o{bt}_{nt}")
                nc.scalar.activation(o[:], ps_ap, AF.Copy)
                o_sb[bt][nt] = o

            # ---- bt=0: six nt-groups + the two ssq accumulators stream ------
            # kt-by-kt as each (s[kt], x[kt][0]) pair lands.  ssq uses the
            # ones COLUMN as the stationary operand (1-column weight load).
            ssq_ps = [
                psum.tile([1, 512], F32, tag="ps", name=f"ssq_ps{h}")
                for h in range(2)
            ]
            ps_g0 = [
                psum.tile([128, 512], F32, tag="ps", name=f"ps0_{nt}")
                for nt in range(6)
            ]
            for kt in range(KT):
                for nt in range(6):
                    main_mm(ps_g0[nt][:], kt, nt, 0)
                for h in range(2):
                    nc.tensor.matmul(
                        ssq_ps[h][:], ones_r[:],
                        s2_sb[kt][:, h * 512:(h + 1) * 512],
                        start=(kt == 0), stop=(kt == KT - 1),
                    )

            ssq_sb = small.tile([1, NS], F32)
            for h in range(2):
                nc.vector.tensor_copy(
                    ssq_sb[0:1, h * 512:(h + 1) * 512], ssq_ps[h][:]
                )
            str_sb = small.tile([128, NT], F32)

            for nt in range(6):
                drain(0, nt, ps_g0[nt][:])
            # bt=0 groups 6,7 run once the ssq accumulators free their banks
            ps_g67 = [
                psum.tile([128, 512], F32, tag="ps", name=f"ps0_{nt}")
                for nt in (6, 7)
            ]
            for i, nt in enumerate((6, 7)):
                for kt in range(KT):
                    main_mm(ps_g67[i][:], kt, nt, 0)
            for i, nt in enumerate((6, 7)):
                drain(0, nt, ps_g67[i][:])

            # ---- bt = 1..3 --------------------------------------------------
            comb = None
            for bt in range(1, BT):
                if bt == 1:
                    # ALL remaining squares + the full magnitude chain are
                    # emitted here, ahead of every PE-gated drain in the ACT
                    # stream -- so `comb` resolves as soon as the data
                    # arrives (~50us), not after the PE finishes bt2
                    for kt in range(KT):
                        square_x(kt, 1)
                    nc.scalar.dma_start(
                        out=ssq_dram[:],
                        in_=ssq_sb[0:1, :].rearrange("o (t p) -> o t p", p=128),
                    )
                    nc.scalar.dma_start(
                        out=str_sb[:], in_=ssq_dram.rearrange("t p -> p t")
                    )
                    for kt in range(KT):
                        square_x(kt, 2)
                    for kt in range(KT):
                        square_x(kt, 3)
                    smax = small.tile([128, NT], F32)
                    nc.vector.tensor_scalar_max(smax[:], str_sb[:], EPS)
                    s_sqrt = small.tile([128, NT], F32)
                    nc.scalar.activation(s_sqrt[:], smax[:], AF.Sqrt)
                    s_seed = small.tile([128, NT], F32)
                    nc.vector.reciprocal(s_seed[:], s_sqrt[:])
                    srs = _newton_rsqrt(
                        nc, small, smax[:], s_seed[:], [128, NT], "srs"
                    )
                    xsum = small.tile([128, 1], F32)
                    nc.vector.tensor_reduce(
                        xsum[:], accs[:], axis=mybir.AxisListType.X, op=ALU.add
                    )
                    xbc = small.tile([128, 1], F32)
                    nc.gpsimd.partition_all_reduce(
                        xbc[:], xsum[:], channels=128,
                        reduce_op=bass_isa.ReduceOp.add,
                    )
                    xmax = small.tile([128, 1], F32)
                    nc.vector.tensor_scalar_max(xmax[:], xbc[:], EPS)
                    x_sqrt = small.tile([128, 1], F32)
                    nc.scalar.activation(x_sqrt[:], xmax[:], AF.Sqrt)
                    x_seed = small.tile([128, 1], F32)
                    nc.vector.reciprocal(x_seed[:], x_sqrt[:])
                    xrs = _newton_rsqrt(
                        nc, small, xmax[:], x_seed[:], [128, 1], "xrs"
                    )
                    # combined per-(partition, nt) scale = support_mag * x_mag
                    comb = small.tile([128, NT], F32)
                    nc.vector.tensor_scalar(
                        comb[:], srs, xrs[:, 0:1], None, op0=ALU.mult
                    )
                ps_g = [
                    psum.tile([128, 512], F32, tag="ps", name=f"ps{bt}_{nt}")
                    for nt in range(NT)
                ]
                if bt == BT - 1:
                    for nt in range(NT):
                        for kt in range(KT):
                            main_mm(ps_g[nt][:], kt, nt, bt)
                else:
                    for kt in range(KT):
                        for nt in range(NT):
                            main_mm(ps_g[nt][:], kt, nt, bt)
                for nt in range(NT):
                    if bt >= 2:
                        # comb resolves (~52us) before these PE-gated drains
                        # execute: fuse the full scale and store directly
                        o = op.tile([128, 512], F32, tag="o", name=f"o{bt}_{nt}")
                        nc.scalar.activation(
                            o[:], ps_g[nt][:], AF.Copy, scale=comb[:, nt:nt + 1]
                        )
                        nc.sync.dma_start(
                            out=o_dram[nt * 128:(nt + 1) * 128,
                                       bt * 512:(bt + 1) * 512],
                            in_=o[:],
                        )
                    else:
                        drain(bt, nt, ps_g[nt][:])

            # ---- second pass: combined scale + store (bt 0..1) --------------
            for bt in range(2):
                for nt in range(NT):
                    o = o_sb[bt][nt]
                    nc.vector.tensor_scalar(
                        o[:], o[:], comb[:, nt:nt + 1], None, op0=ALU.mult
                    )
                    nc.sync.dma_start(
                        out=o_dram[nt * 128:(nt + 1) * 128, bt * 512:(bt + 1) * 512],
                        in_=o[:],
                    )
    nc.compile()
    return nc


_NC_CACHE = []


def _get_nc():
    if not _NC_CACHE:
        _NC_CACHE.append(build_nc())
    return _NC_CACHE[0]


def kernel(support_set: np.ndarray, input_image: np.ndarray) -> np.ndarray:
    support_set = np.asarray(support_set, dtype=np.float32)
    input_image = np.asarray(input_image, dtype=np.float32)
    assert support_set.shape == (N_CORES * NS, D)
    assert input_image.shape == (B, D)

    s_t = np.ascontiguousarray(support_set.T)  # [1024, 8192]
    x_t = np.ascontiguousarray(input_image.T)  # [1024, 2048]
    in_maps = [
        {
            "s_t": np.ascontiguousarray(s_t[:, i * NS:(i + 1) * NS]),
            "x_t": x_t,
        }
        for i in range(N_CORES)
    ]
    nc = _get_nc()
    res = run_bass_kernel_spmd(nc, in_maps, core_ids=list(range(N_CORES)))
    global LAST_RESULT
    LAST_RESULT = res
    out = np.concatenate([res.results[i]["out"] for i in range(N_CORES)], axis=0)
    return out[:, :, None]


LAST_RESULT = None

